# revision 1
# baseline (speedup 1.0000x reference)
"""Trainium2 Bass kernel for nn_Aggregator (gnn_message_passing).

pooled[B,D] = owner_masks.f32 @ ((nodes@Wt + bt) * sigmoid(nodes@Wg + bg))

Sharding: nodes (and owner_masks columns) split along N across 8 cores.
Each core computes a partial [B, 2D] = [M@(A*G) | M@G]; the host sums the
8 partials and applies the bt column bias algebraically:
    pooled = sum_c pool1_c + (sum_c pool2_c) * bt[None, :]
(exact: (A + 1 bt^T) * G = A*G + (1 bt^T)*G and M @ ((1 bt^T)*G) =
(M@G) diag(bt)).

Device pipeline (per core; fp16 inputs, fp32 accumulation). DMA loads come
in multi-chunk slabs (few, large HWDGE dispatches); compute runs per chunk
of 8 128-node tiles:
    PE : MM(psum_d[:,sl], lhsT=nodesT[:,sl], rhs=Wt)  (A tile, [n,D])
         MM(psum_g[:,sl], lhsT=nodesT[:,sl], rhs=Wg)  (G_pre,  [n,D])
    DVE: gpre = psum_g + bg_bcast          (fp16 out)
    ACT: mg[:, :, :D]  = copy(psum_d)      (fp16)
    ACT: mg[:, :, D:]  = sigmoid(gpre)     (fp16)
    DVE: mg[:, :, :D] *= mg[:, :, D:]      (in-place msg = A*G)
    PE : pool12[B, 2D] += masksT_t.T @ mg_t   (one 256-wide MM per tile)
Cost-model (TimelineSim) exec time: ~141.6 us/core; engine busy: ACT ~129 us
(sigmoid + psum_d eviction — the bottleneck), DVE ~111 us (bias-add + mul),
PE ~106 us (MAC floor for this algorithm), DMA data ~92 us (fp16 floor),
HWDGE ~24 us. Startup is trimmed by emitting the first node slab right
after wt on the DMA ring and burning the PE HAM clock ramp with 6 warm-up
matmuls during the initial DMA wait. Structure notes from iteration:
keeping copy_d wholly on ACT beats any ACT/DVE alternation or
DVE-psum-fused multiply (those extend psum_d slot lifetime and stall the
3-slot PSUM rotation); GPSIMD tensor ops (0.42 efficiency) rate-limit the
chunk pipeline if placed on the msg path; mask DMAs must stay on the SP
HWDGE ring.
"""

import json

import numpy as np

import concourse.bass as bass
import concourse.mybir as mybir
import concourse.tile as tile
from concourse import bass2jax as _b2j
from concourse import bass_utils as _bu
from concourse.bass_utils import run_bass_kernel_spmd


def _split_excess_waits_json(bir_json) -> bytes:
    """Walrus in this container accepts at most 1 embedded sem-wait per
    instruction (2 for EventSemaphore). Tile emits instructions (notably the
    kernel-tail Drain) with more. Move excess waits onto injected
    EventSemaphore instructions placed immediately before the offender in
    the same engine stream — identical blocking semantics."""
    if isinstance(bir_json, str):
        bir_json = bir_json.encode()
    d = json.loads(bir_json)
    counter = [0]

    def fix_block(b):
        new = []
        for inst in b.get("instructions", []):
            si = inst.get("sync_info")
            waits = (si or {}).get("on_wait") or []
            cap = 2 if inst.get("opcode") == "EventSemaphore" else 1
            if len(waits) > cap:
                keep, excess = waits[:cap], waits[cap:]
                for j in range(0, len(excess), 2):
                    counter[0] += 1
                    new.append(
                        {
                            "debug": inst.get("debug"),
                            "engine": inst["engine"],
                            "ins": [],
                            "outs": [],
                            "name": f"antsplit_ev_{counter[0]}",
                            "opcode": "EventSemaphore",
                            "sync_info": {
                                "on_update": [],
                                "on_wait": excess[j : j + 2],
                            },
                        }
                    )
                si["on_wait"] = keep
            new.append(inst)
        b["instructions"] = new
        for sb in b.get("blocks", []):
            fix_block(sb)

    for f in d.get("functions", []):
        for blk in f.get("blocks", []):
            fix_block(blk)
    return json.dumps(d).encode()


if not getattr(_bu, "_ant_split_waits_patched", False):
    _orig_compile_bir_kernel = _bu.compile_bir_kernel

    def _patched_compile_bir_kernel(bir_json, tmpdir, neff_name="file.neff"):
        return _orig_compile_bir_kernel(
            _split_excess_waits_json(bir_json), tmpdir, neff_name
        )

    _bu.compile_bir_kernel = _patched_compile_bir_kernel
    _b2j.compile_bir_kernel = _patched_compile_bir_kernel
    _bu._ant_split_waits_patched = True

N_CORES = 8
N_TOTAL = 500_000
B = 128
S = 128
D = 128
P = 128

N_PER_CORE = N_TOTAL // N_CORES          # 62500
TILES_PER_CHUNK = 8
CHUNK = TILES_PER_CHUNK * P              # 1024
# small first slabs so compute starts early; 62 chunks = 63488 nodes total
SLAB_CHUNKS = [2, 2] + [4] * 14 + [2]
N_CHUNKS = sum(SLAB_CHUNKS)              # 62
N_TILES = N_CHUNKS * TILES_PER_CHUNK     # 496
N_PAD = N_TILES * P                      # 63488

F16 = mybir.dt.float16
F32 = mybir.dt.float32
NP_F16 = np.float16


# bisect/debug switches
OPTS = {
    "touches": True,     # one-time const touch ops
    "sigmoid": True,     # False -> plain Copy instead of Sigmoid
    "mm2": True,         # False -> skip the pooling matmuls (evict psum_d instead)
    "mm1": True,         # False -> skip the feature matmuls entirely
    "elemwise": True,    # False -> skip DVE/ACT elementwise ops
    # chunks whose sigma/mul run as one wide SBUF op. 1 is optimal: larger
    # groups amortize ACT per-op init but the sigma lump stalls the 3-slot
    # PSUM rotation (8-bank budget) and regresses end-to-end.
    "group": 1,
    # trailing tiles of each chunk's psum_d eviction done on DVE instead of
    # ACT. 0 is optimal: any DVE share extends the TT->copy->mul chain and
    # psum_d's lifetime, stalling the PSUM rotation (same failure mode as
    # every other copy_d offload variant).
    "dve_copy_tiles": 0,
    # dummy PE matmuls at kernel start to burn the HAM clock ramp while the
    # first node slab is still in flight
    "warm_mms": 6,
    # emit the first node-slab DMA right after wt (before wg/bgb consts)
    "early_nod0": True,
    # process the first/last chunk in two halves to shorten the serial
    # dependency chain at the pipeline edges. False is optimal: the extra
    # per-op access-latency inits outweigh the halved edge chain.
    "edge_halves": False,
}


def build_bass() -> bass.Bass:
    nc = bass.Bass()

    nodesT = nc.dram_tensor("nodesT", [P, N_PAD], F16, kind="ExternalInput").ap()
    masksT = nc.dram_tensor("masksT", [P, N_TILES, B], F16, kind="ExternalInput").ap()
    wt_d = nc.dram_tensor("wt", [S, D], F16, kind="ExternalInput").ap()
    wg_d = nc.dram_tensor("wg", [S, D], F16, kind="ExternalInput").ap()
    bgb_d = nc.dram_tensor("bgb", [P, CHUNK], F32, kind="ExternalInput").ap()
    out_d = nc.dram_tensor("out", [B, 2 * D], F32, kind="ExternalOutput").ap()

    with tile.TileContext(nc) as tc:
        with (
            tc.tile_pool(name="consts", bufs=1) as consts,
            tc.tile_pool(name="scratch", bufs=1) as scratch,
            tc.tile_pool(name="nodes", bufs=4) as nodes_pool,
            tc.tile_pool(name="masks", bufs=4) as masks_pool,
            tc.tile_pool(name="gpre", bufs=3) as gpre_pool,
            tc.tile_pool(name="dt", bufs=3) as d_pool,
            tc.tile_pool(name="gt", bufs=3) as g_pool,
            tc.tile_pool(name="outs", bufs=1) as out_pool,
            tc.tile_pool(name="ps", bufs=3, space="PSUM") as ps_pool,
            tc.tile_pool(name="acc", bufs=1, space="PSUM") as acc_pool,
        ):
            def emit_slab(s_chunks, slab_off):
                slab_n = s_chunks * CHUNK
                nod_slab = nodes_pool.tile([P, 4 * CHUNK], F16, tag="nod_slab")
                nc.sync.dma_start(
                    nod_slab[:, :slab_n],
                    nodesT[:, slab_off : slab_off + slab_n],
                )
                mk_slab = masks_pool.tile(
                    [P, 4 * TILES_PER_CHUNK, B], F16, tag="mk_slab"
                )
                to = slab_off // P
                nc.sync.dma_start(
                    mk_slab[:, : s_chunks * TILES_PER_CHUNK, :],
                    masksT[:, to : to + s_chunks * TILES_PER_CHUNK, :],
                )
                return nod_slab, mk_slab

            wt_sb = consts.tile([S, D], F16)
            nc.sync.dma_start(wt_sb[:], wt_d)
            if OPTS["early_nod0"]:
                # put the first node slab on the DMA ring right after wt so
                # its (large) transfer overlaps the remaining const loads
                slab_n0 = SLAB_CHUNKS[0] * CHUNK
                nod_slab0 = nodes_pool.tile([P, 4 * CHUNK], F16, tag="nod_slab")
                nc.sync.dma_start(nod_slab0[:, :slab_n0], nodesT[:, :slab_n0])
            wg_sb = consts.tile([S, D], F16)
            nc.sync.dma_start(wg_sb[:], wg_d)
            bgb_sb = consts.tile([P, CHUNK], F32)
            nc.sync.dma_start(bgb_sb[:], bgb_d)

            # One-time const touches: absorb the const-DMA semaphores into
            # each engine's observed clock so hot-loop instructions never
            # need a second (DMA) wait slot.
            if OPTS["touches"]:
                dve_scratch = scratch.tile([1, 2], F32)
                nc.vector.tensor_copy(out=dve_scratch[:1, :1], in_=bgb_sb[:1, :1])
                nc.tensor.ldweights(wt_sb[:, :1])
                nc.tensor.ldweights(wg_sb[:, :1])
            if OPTS["warm_mms"]:
                # burn the PE HAM clock ramp during the initial DMA wait;
                # scratch lives in the rotating psum pool (one-time slot use)
                warm_ps = ps_pool.tile([P, CHUNK], F32, tag="ps")
                for _ in range(OPTS["warm_mms"]):
                    nc.tensor.matmul(
                        warm_ps[:, :D], wt_sb[:], wg_sb[:], start=True, stop=True
                    )
                nc.vector.tensor_copy(
                    out=dve_scratch[:1, 1:2], in_=warm_ps[:1, :1]
                )
            if OPTS["early_nod0"]:
                mk_slab0 = masks_pool.tile(
                    [P, 4 * TILES_PER_CHUNK, B], F16, tag="mk_slab"
                )
                nc.sync.dma_start(
                    mk_slab0[:, : SLAB_CHUNKS[0] * TILES_PER_CHUNK, :],
                    masksT[:, : SLAB_CHUNKS[0] * TILES_PER_CHUNK, :],
                )
                slab0 = (nod_slab0, mk_slab0)
            else:
                slab0 = emit_slab(SLAB_CHUNKS[0], 0)

            if OPTS["mm2"]:
                # pool12[:, :D] accumulates M@(A*G); [:, D:] accumulates M@G
                pool12 = acc_pool.tile([B, 2 * D], F32)

            c = 0
            slab_off = 0
            for s, s_chunks in enumerate(SLAB_CHUNKS):
                if s == 0:
                    nod_slab, mk_slab = slab0
                else:
                    nod_slab, mk_slab = emit_slab(s_chunks, slab_off)
                slab_off += s_chunks * CHUNK

                for cs in range(s_chunks):
                    nod = nod_slab[:, cs * CHUNK : (cs + 1) * CHUNK]
                    mk = mk_slab[
                        :, cs * TILES_PER_CHUNK : (cs + 1) * TILES_PER_CHUNK, :
                    ]

                    if not OPTS["mm1"]:
                        nc.vector.tensor_copy(
                            out=dve_scratch[:1, :1], in_=nod[:1, :1]
                        )
                        nc.vector.tensor_copy(
                            out=dve_scratch[:1, 1:2], in_=mk[:1, 0, :1]
                        )
                        c += 1
                        continue

                    psum_d = ps_pool.tile([P, CHUNK], F32, tag="ps")
                    psum_g = ps_pool.tile([P, CHUNK], F32, tag="ps")
                    for t in range(TILES_PER_CHUNK):
                        sl = bass.ts(t, P)
                        nc.tensor.matmul(
                            psum_d[:, sl], nod[:, sl], wt_sb[:], start=True, stop=True
                        )
                        nc.tensor.matmul(
                            psum_g[:, sl], nod[:, sl], wg_sb[:], start=True, stop=True
                        )

                    if not OPTS["elemwise"]:
                        nc.vector.tensor_copy(
                            out=dve_scratch[:1, :1], in_=psum_d[:1, :1]
                        )
                        nc.vector.tensor_copy(
                            out=dve_scratch[:1, 1:2], in_=psum_g[:1, :1]
                        )
                        nc.vector.tensor_copy(
                            out=dve_scratch[:1, 1:2], in_=mk[:1, 0, :1]
                        )
                        c += 1
                        continue

                    G = OPTS["group"]
                    TPC = TILES_PER_CHUNK
                    if (
                        OPTS["edge_halves"]
                        and G == 1
                        and OPTS["dve_copy_tiles"] == 0
                        and (c == 0 or c == N_CHUNKS - 1)
                    ):
                        # first/last chunk: process in two 4-tile halves so
                        # the serial mm1->TT->sigma->mul->mm2 chain at the
                        # pipeline edges is half as deep (sub-tile deps let
                        # each half start as soon as its mm1s finish)
                        gpre_e = gpre_pool.tile([P, G, CHUNK], F16, tag="gpre")
                        mg_e = d_pool.tile([P, G * TPC, 2 * D], F16, tag="mg")
                        pd3 = psum_d.rearrange("p (t d) -> p t d", d=D)
                        ht = TPC // 2
                        for h in range(2):
                            tlo, thi = h * ht, (h + 1) * ht
                            w0, w1 = tlo * P, thi * P
                            nc.vector.tensor_add(
                                out=gpre_e[:, 0, w0:w1],
                                in0=psum_g[:, w0:w1],
                                in1=bgb_sb[:, w0:w1],
                            )
                            nc.scalar.copy(
                                out=mg_e[:, tlo:thi, :D], in_=pd3[:, tlo:thi, :]
                            )
                            nc.scalar.activation(
                                mg_e[:, tlo:thi, D:],
                                gpre_e[:, 0, w0:w1].rearrange(
                                    "p (t d) -> p t d", d=D
                                ),
                                mybir.ActivationFunctionType.Sigmoid
                                if OPTS["sigmoid"]
                                else mybir.ActivationFunctionType.Copy,
                            )
                            nc.vector.tensor_mul(
                                out=mg_e[:, tlo:thi, :D],
                                in0=mg_e[:, tlo:thi, :D],
                                in1=mg_e[:, tlo:thi, D:],
                            )
                            if OPTS["mm2"]:
                                for t in range(tlo, thi):
                                    first = c == 0 and t == 0
                                    last = c == N_CHUNKS - 1 and t == TPC - 1
                                    nc.tensor.matmul(
                                        pool12[:],
                                        mk[:, t, :],
                                        mg_e[:, t, :],
                                        start=first,
                                        stop=last,
                                        skip_group_check=True,
                                    )
                        c += 1
                        continue
                    gi = c % G
                    if gi == 0:
                        gsize = min(G, N_CHUNKS - c)
                        # group-wide buffers: sigma and the multiply run once
                        # per group to amortize per-op access-latency init
                        gpre_t = gpre_pool.tile([P, G, CHUNK], F16, tag="gpre")
                        mg_t = d_pool.tile([P, G * TPC, 2 * D], F16, tag="mg")
                        pending_mk = []

                    nc.vector.tensor_add(
                        out=gpre_t[:, gi, :], in0=psum_g[:], in1=bgb_sb[:]
                    )
                    psum_d3 = psum_d.rearrange("p (t d) -> p t d", d=D)
                    dct = OPTS["dve_copy_tiles"]
                    split = TPC - dct
                    if split:
                        nc.scalar.copy(
                            out=mg_t[:, gi * TPC : gi * TPC + split, :D],
                            in_=psum_d3[:, :split, :],
                        )
                    if dct:
                        nc.vector.tensor_copy(
                            out=mg_t[:, gi * TPC + split : (gi + 1) * TPC, :D],
                            in_=psum_d3[:, split:, :],
                        )
                    pending_mk.append(mk)

                    if gi == gsize - 1:
                        nt = gsize * TPC
                        nc.scalar.activation(
                            mg_t[:, :nt, D:],
                            gpre_t[:, :gsize, :].rearrange(
                                "p g (t d) -> p (g t) d", d=D
                            ),
                            mybir.ActivationFunctionType.Sigmoid
                            if OPTS["sigmoid"]
                            else mybir.ActivationFunctionType.Copy,
                        )
                        nc.vector.tensor_mul(
                            out=mg_t[:, :nt, :D],
                            in0=mg_t[:, :nt, :D],
                            in1=mg_t[:, :nt, D:],
                        )
                        if OPTS["mm2"]:
                            for pi, pmk in enumerate(pending_mk):
                                cc = c - gsize + 1 + pi
                                for t in range(TPC):
                                    first = cc == 0 and t == 0
                                    last = (
                                        cc == N_CHUNKS - 1 and t == TPC - 1
                                    )
                                    nc.tensor.matmul(
                                        pool12[:],
                                        pmk[:, t, :],
                                        mg_t[:, pi * TPC + t, :],
                                        start=first,
                                        stop=last,
                                        skip_group_check=True,
                                    )
                        else:
                            nc.vector.tensor_copy(
                                out=dve_scratch[:1, :2], in_=mg_t[:1, 0, :2]
                            )
                    c += 1

            res = out_pool.tile([B, 2 * D], F32)
            if OPTS["mm2"]:
                # DVE finishes before ACT at the tail; evicting there starts
                # the output DMA sooner
                nc.vector.tensor_copy(out=res[:], in_=pool12[:])
            else:
                nc.vector.tensor_copy(out=res[:1, :2], in_=dve_scratch[:1, :2])
            nc.sync.dma_start(out_d, res[:])

    return nc


_CACHE: dict = {}


def _get_bass() -> bass.Bass:
    if "nc" not in _CACHE:
        _CACHE["nc"] = build_bass()
    return _CACHE["nc"]


def _prepare_in_maps(nodes, owner_masks, Wt, bt, Wg, bg):
    nodes_h = np.asarray(nodes, dtype=NP_F16)
    masks = np.asarray(owner_masks)
    wt_h = np.ascontiguousarray(np.asarray(Wt, dtype=NP_F16))
    wg_h = np.ascontiguousarray(np.asarray(Wg, dtype=NP_F16))
    bg32 = np.asarray(bg, dtype=np.float32)
    bgb = np.ascontiguousarray(
        np.tile(bg32[None, :], (P, CHUNK // D)).reshape(P, CHUNK)
    )

    in_maps = []
    for core in range(N_CORES):
        off = core * N_PER_CORE
        ncr = np.zeros((P, N_PAD), dtype=NP_F16)
        ncr[:, :N_PER_CORE] = nodes_h[off : off + N_PER_CORE].T
        mp = np.zeros((B, N_PAD), dtype=NP_F16)
        mp[:, :N_PER_CORE] = masks[:, off : off + N_PER_CORE]
        mkt = np.ascontiguousarray(mp.reshape(B, N_TILES, P).transpose(2, 1, 0))
        in_maps.append(
            {
                "nodesT": ncr,
                "masksT": mkt,
                "wt": wt_h,
                "wg": wg_h,
                "bgb": bgb,
            }
        )
    return in_maps


def run(inputs: dict, trace: bool = False):
    """Run the kernel. Returns (pooled [B, D] float32, BassKernelResults)."""
    nc = _get_bass()
    in_maps = _prepare_in_maps(**inputs)
    rb = run_bass_kernel_spmd(
        nc, in_maps, core_ids=list(range(N_CORES)), trace=trace
    )
    parts = np.stack([r["out"].astype(np.float64) for r in rb.results])
    tot = parts.sum(axis=0)
    bt64 = np.asarray(inputs["bt"], dtype=np.float64)
    pooled = tot[:, :D] + tot[:, D:] * bt64[None, :]
    return pooled.astype(np.float32), rb


def kernel(**inputs) -> np.ndarray:
    try:
        out, _ = run(inputs, trace=False)
    except Exception:
        # transient device errors (e.g. residual bad state from a previous
        # crashed NEFF) have been observed once; one retry clears them
        out, _ = run(inputs, trace=False)
    return out


if __name__ == "__main__":
    rng = np.random.default_rng(0)
    demo = {
        "nodes": rng.standard_normal((N_TOTAL, S), dtype=np.float32),
        "owner_masks": rng.integers(0, 2, (B, N_TOTAL)).astype(np.int32),
        "Wt": rng.standard_normal((S, D), dtype=np.float32) * 0.09,
        "bt": rng.standard_normal(D).astype(np.float32) * 0.09,
        "Wg": rng.standard_normal((S, D), dtype=np.float32) * 0.09,
        "bg": rng.standard_normal(D).astype(np.float32) * 0.09,
    }
    out = kernel(**demo)
    print(out.shape, out.dtype, np.abs(out).mean())



# revision 2
# speedup vs baseline: 4.3421x; 4.3421x over previous
"""Trainium2 Bass kernel for nn_Aggregator (gnn_message_passing), v2.

pooled[B,D] = owner_masks.f32 @ ((nodes@Wt + bt) * sigmoid(nodes@Wg + bg))

Sharding: nodes (and owner_masks columns) split along N across 8 cores;
host sums the per-core [B, D] partials.

v2 design (vs v1's node-partition layout): mm1 runs TRANSPOSED
(stationary = W, psum_dT/gT are [D, nodes]) so both biases fold into
per-partition scalar operands:
  ACT : gatesT = sigmoid(psum_gT + bg)            (bias fused, 1 pass)
  DVE : msgT   = (psum_dT + bt) * gatesT          (scalar_tensor_tensor)
mm2 then needs msg back in [node, D]: PE transposes it per 128-tile
(fp16 -> fp16 PSUM) and ACT/DVE evict-copy pairs of chunks back to SBUF
(one wide op per 2 chunks amortizes per-op access-latency init; fp16
PSUM reads earn DVE's 2x_1p mode). Masks are staged in fp8e4 (0/1
exact; fp8-stationary x fp16-moving matmul verified on HW) halving mask
DMA. mm2 is D-wide only (no [B,2D] trick, no host bias fix-up).

Schedule: chunks of 512 nodes; per-index emission order is mm2(c-MD),
transpose(pair at c-TD), front(c) so every engine's in-order queue sees
oldest-deps-first; psum_dT/gT rotate through 5 single-bank PSUM slots,
pair transposes through 2 banks, the [B,D] accumulator holds the last
bank. Slab input DMAs prefetch one slab ahead on the SP ring.
Cost-model: 121.3us/core (PE-bound: 878ns/chunk = mm1 426 + transpose
213 + mm2 213 + decode; ACT 820, DVE 855, DMA 546). Rejected variants
(all sim-verified worse): DMA XBAR transposes in any mix (their ~2.5us
round trip + DMA-instruction SEQ-holds starve the pipeline, and PE
idling drops its pstate clock); ACT evicts beyond ~40%% (delays the
sacred mm1_g->sigmoid->stt chain); un-paired evicts; [B,2D] mm2.
"""

import json

import numpy as np
import ml_dtypes

import concourse.bass as bass
import concourse.mybir as mybir
import concourse.tile as tile
from concourse import bass2jax as _b2j
from concourse import bass_utils as _bu
from concourse.bass_utils import run_bass_kernel_spmd


def _split_excess_waits_json(bir_json) -> bytes:
    """Walrus in this container accepts at most 1 embedded sem-wait per
    instruction (2 for EventSemaphore). Tile emits instructions (notably the
    kernel-tail Drain) with more. Move excess waits onto injected
    EventSemaphore instructions placed immediately before the offender in
    the same engine stream — identical blocking semantics."""
    if isinstance(bir_json, str):
        bir_json = bir_json.encode()
    d = json.loads(bir_json)
    counter = [0]

    def fix_block(b):
        new = []
        for inst in b.get("instructions", []):
            si = inst.get("sync_info")
            waits = (si or {}).get("on_wait") or []
            cap = 2 if inst.get("opcode") == "EventSemaphore" else 1
            if len(waits) > cap:
                keep, excess = waits[:cap], waits[cap:]
                for j in range(0, len(excess), 2):
                    counter[0] += 1
                    new.append(
                        {
                            "debug": inst.get("debug"),
                            "engine": inst["engine"],
                            "ins": [],
                            "outs": [],
                            "name": f"antsplit_ev_{counter[0]}",
                            "opcode": "EventSemaphore",
                            "sync_info": {
                                "on_update": [],
                                "on_wait": excess[j : j + 2],
                            },
                        }
                    )
                si["on_wait"] = keep
            new.append(inst)
        b["instructions"] = new
        for sb in b.get("blocks", []):
            fix_block(sb)

    for f in d.get("functions", []):
        for blk in f.get("blocks", []):
            fix_block(blk)
    return json.dumps(d).encode()


if not getattr(_bu, "_ant_split_waits_patched", False):
    _orig_compile_bir_kernel = _bu.compile_bir_kernel

    def _patched_compile_bir_kernel(bir_json, tmpdir, neff_name="file.neff"):
        return _orig_compile_bir_kernel(
            _split_excess_waits_json(bir_json), tmpdir, neff_name
        )

    _bu.compile_bir_kernel = _patched_compile_bir_kernel
    _b2j.compile_bir_kernel = _patched_compile_bir_kernel
    _bu._ant_split_waits_patched = True

N_CORES = 8
N_TOTAL = 500_000
B = 128
S = 128
D = 128
P = 128

N_PER_CORE = N_TOTAL // N_CORES          # 62500
TILES_PER_CHUNK = 4
CHUNK = TILES_PER_CHUNK * P              # 512
# slabs (in chunks): small first slabs so compute starts early
SLAB_CHUNKS = [2, 2, 4] + [8] * 14 + [4]
N_CHUNKS = sum(SLAB_CHUNKS)              # 124
N_TILES = N_CHUNKS * TILES_PER_CHUNK     # 496
N_PAD = N_TILES * P                      # 63488

F16 = mybir.dt.float16
F32 = mybir.dt.float32
F8 = mybir.dt.float8e4
NP_F16 = np.float16
NP_F8 = ml_dtypes.float8_e4m3

OPTS = {
    # pair index -> transpose mode: every dma_t_period-th PAIR of chunks is
    # transposed by the DMA XBAR (one instruction per pair), the rest on PE.
    # 0 disables DMA transposes.
    "dma_t_period": 0,
    # PE-pair evicts whose index mod 5 is in this set run on ACT, the rest
    # on DVE (empty = all DVE). ~40% ACT balances ACT (sigmoid-laden)
    # against DVE (stt-laden).
    "act_evict_phases": (1, 3),
    "warm_mms": 6,
    "masks_fp8": True,
    "touches": True,
    # software pipelining: in PE program order, the transpose of pair p is
    # emitted TD chunks after p's second chunk, and chunk c's mm2 MD chunks
    # after c, so their cross-engine deps (ACT sigmoid -> DVE stt ->
    # [transpose+evict | DMA transpose]) are resolved before PE's in-order
    # queue reaches them. DMA-pair mm2s get a larger deadline (md_dma) than
    # PE-pair mm2s (md): the XBAR round trip is ~2.5us, and emitting those
    # mm2s in strict chunk order would park ready PE-pair work behind them
    # in PE's in-order queue.
    "td": 2,
    "md": 8,
    "md_dma": 14,
}


def build_bass() -> bass.Bass:
    nc = bass.Bass()

    nodesT = nc.dram_tensor("nodesT", [P, N_PAD], F16, kind="ExternalInput").ap()
    mk_dt = F8 if OPTS["masks_fp8"] else F16
    masksT = nc.dram_tensor(
        "masksT", [P, N_TILES, B], mk_dt, kind="ExternalInput"
    ).ap()
    # packed constants: one fp16 [S, 3D] = [Wg | Wt | I], one fp32 [P, 2] =
    # [bt | bg] — two HWDGE dispatches instead of six at startup
    cst16_d = nc.dram_tensor("cst16", [S, 3 * D], F16, kind="ExternalInput").ap()
    cst32_d = nc.dram_tensor("cst32", [P, 2], F32, kind="ExternalInput").ap()
    out_d = nc.dram_tensor("out", [B, D], F32, kind="ExternalOutput").ap()

    with tile.TileContext(nc) as tc:
        with (
            tc.tile_pool(name="consts", bufs=1) as consts,
            tc.tile_pool(name="scratch", bufs=1) as scratch,
            tc.tile_pool(name="nodes", bufs=3) as nodes_pool,
            tc.tile_pool(name="masks", bufs=3) as masks_pool,
            tc.tile_pool(name="gT", bufs=3) as g_pool,
            tc.tile_pool(name="msgT", bufs=10) as mt_pool,
            tc.tile_pool(name="msg", bufs=14) as m_pool,
            tc.tile_pool(name="outs", bufs=1) as out_pool,
            tc.tile_pool(name="ps", bufs=5, space="PSUM") as ps_pool,
            tc.tile_pool(name="pm", bufs=2, space="PSUM") as pm_pool,
            tc.tile_pool(name="acc", bufs=1, space="PSUM") as acc_pool,
        ):
            MAX_SLAB = max(SLAB_CHUNKS)

            def emit_slab(s_chunks, slab_off):
                slab_n = s_chunks * CHUNK
                nod_slab = nodes_pool.tile([P, MAX_SLAB * CHUNK], F16, tag="nod")
                nc.sync.dma_start(
                    nod_slab[:, :slab_n],
                    nodesT[:, slab_off : slab_off + slab_n],
                )
                mk_slab = masks_pool.tile(
                    [P, MAX_SLAB * TILES_PER_CHUNK, B], mk_dt, tag="mk"
                )
                to = slab_off // P
                nc.sync.dma_start(
                    mk_slab[:, : s_chunks * TILES_PER_CHUNK, :],
                    masksT[:, to : to + s_chunks * TILES_PER_CHUNK, :],
                )
                return nod_slab, mk_slab

            cst16 = consts.tile([S, 3 * D], F16)
            nc.sync.dma_start(cst16[:], cst16_d)
            # first node slab right after the consts: its (large) transfer
            # overlaps the scalar-const load and the warm-up matmuls
            slab_n0 = SLAB_CHUNKS[0] * CHUNK
            nod_slab0 = nodes_pool.tile([P, MAX_SLAB * CHUNK], F16, tag="nod")
            nc.sync.dma_start(nod_slab0[:, :slab_n0], nodesT[:, :slab_n0])
            cst32 = consts.tile([P, 2], F32)
            nc.sync.dma_start(cst32[:], cst32_d)
            wg_sb = cst16[:, 0:D]
            wt_sb = cst16[:, D : 2 * D]
            id_sb = cst16[:, 2 * D : 3 * D]
            bt_sb = cst32[:, 0:1]
            bg_sb = cst32[:, 1:2]

            # One-time const touches: absorb the const-DMA semaphores into
            # each engine's observed clock so hot-loop instructions never
            # need a second (DMA) wait slot.
            if OPTS["touches"]:
                dve_scratch = scratch.tile([P, 4], F32)
                nc.vector.tensor_copy(out=dve_scratch[:, :1], in_=bt_sb)
                nc.scalar.copy(out=dve_scratch[:1, 2:3], in_=dve_scratch[:1, :1])
                nc.tensor.ldweights(wt_sb[:, :1])
            if OPTS["warm_mms"]:
                # burn the PE pstate ramp during the initial DMA wait
                warm_ps = ps_pool.tile([P, CHUNK], F32, tag="ps")
                for _ in range(OPTS["warm_mms"]):
                    nc.tensor.matmul(
                        warm_ps[:, :D], wt_sb, wg_sb, start=True, stop=True
                    )
                nc.vector.tensor_copy(
                    out=dve_scratch[:1, 3:4], in_=warm_ps[:1, :1]
                )
            mk_slab0 = masks_pool.tile(
                [P, MAX_SLAB * TILES_PER_CHUNK, B], mk_dt, tag="mk"
            )
            nc.sync.dma_start(
                mk_slab0[:, : SLAB_CHUNKS[0] * TILES_PER_CHUNK, :],
                masksT[:, : SLAB_CHUNKS[0] * TILES_PER_CHUNK, :],
            )

            pooled = acc_pool.tile([B, D], F32)

            TPC = TILES_PER_CHUNK
            dtp = OPTS["dma_t_period"]
            TD, MD = OPTS["td"], OPTS["md"]

            def is_dma_pair(p):
                return dtp and (p % dtp) == dtp - 1

            # per-chunk state kept alive between pipeline stages
            state = {}          # c -> dict(mk=..., pair=...)
            pair_state = {}     # pair idx -> dict(msgT=..., msg=...)
            n_pe_evict = [0]
            n_mm2 = [0]

            # slab schedule keyed by the chunk index ONE SLAB AHEAD: slab
            # s+1's DMAs are emitted when slab s's first chunk is processed,
            # so input transfers always overlap the previous slab's compute.
            slab_info = []
            off = 0
            for s, s_chunks in enumerate(SLAB_CHUNKS):
                slab_info.append((s_chunks, off))
                off += s_chunks * CHUNK
            prefetch_at = {}     # front chunk idx -> slab idx to emit
            acc_c = 0
            for s, s_chunks in enumerate(SLAB_CHUNKS):
                if s + 1 < len(SLAB_CHUNKS):
                    prefetch_at[acc_c] = s + 1
                acc_c += s_chunks
            chunk_slab = []      # chunk idx -> (slab idx, chunk-within-slab)
            for s, s_chunks in enumerate(SLAB_CHUNKS):
                for cs in range(s_chunks):
                    chunk_slab.append((s, cs))
            slabs = {0: (nod_slab0, mk_slab0)}

            def emit_front(c):
                if c in prefetch_at:
                    s = prefetch_at[c]
                    slabs[s] = emit_slab(*slab_info[s])
                s, cs = chunk_slab[c]
                nod_slab, mk_slab = slabs[s]
                nod = nod_slab[:, cs * CHUNK : (cs + 1) * CHUNK]
                mk = mk_slab[:, cs * TPC : (cs + 1) * TPC, :]

                # gate path first: it heads the longest cross-engine chain
                # (mm1_g -> sigmoid -> stt), and psum_dT's WAR slot-recycle
                # (freed by stt) gains slack from mm1_d running second
                psum_gT = ps_pool.tile([P, CHUNK], F32, tag="ps")
                psum_dT = ps_pool.tile([P, CHUNK], F32, tag="ps")
                nc.tensor.matmul(psum_gT[:], wg_sb, nod, start=True, stop=True)
                nc.tensor.matmul(psum_dT[:], wt_sb, nod, start=True, stop=True)

                gT = g_pool.tile([P, CHUNK], F16, tag="g")
                nc.scalar.activation(
                    gT[:],
                    psum_gT[:],
                    mybir.ActivationFunctionType.Sigmoid,
                    bias=bg_sb,
                    scale=1.0,
                )
                # msgT lives in per-PAIR tiles so the transpose+evict (or
                # DMA XBAR transpose) runs once per pair: half the per-op
                # overhead and half the HWDGE dispatches
                pi, half = divmod(c, 2)
                if half == 0:
                    msgT = mt_pool.tile([P, 2 * CHUNK], F16, tag="mt", name="msgT")
                    pair_state[pi] = {"msgT": msgT}
                msgT = pair_state[pi]["msgT"]
                nc.vector.scalar_tensor_tensor(
                    out=msgT[:, half * CHUNK : (half + 1) * CHUNK],
                    in0=psum_dT[:],
                    scalar=bt_sb,
                    in1=gT[:],
                    op0=mybir.AluOpType.add,
                    op1=mybir.AluOpType.mult,
                )
                state[c] = {"mk": mk, "pair": pi}

            def emit_transpose(p):
                st = pair_state[p]
                msgT = st["msgT"]
                msg = m_pool.tile([P, 2 * TPC, P], F16, tag="m")
                if is_dma_pair(p):
                    # SP ring, delayed by TD chunks: DMA instructions hold
                    # the issuing SEQ through their sem waits, so only
                    # dispatch once the msgT waits are already satisfied
                    nc.sync.dma_start_transpose(msg[:], msgT[:])
                else:
                    psum_m = pm_pool.tile([P, 2 * TPC, P], F16, tag="pm")
                    for t in range(2 * TPC):
                        nc.tensor.transpose(
                            psum_m[:, t, :],
                            msgT[:, t * P : (t + 1) * P],
                            id_sb,
                        )
                    if n_pe_evict[0] % 5 in OPTS["act_evict_phases"]:
                        nc.scalar.copy(out=msg[:], in_=psum_m[:])
                    else:
                        nc.vector.tensor_copy(out=msg[:], in_=psum_m[:])
                    n_pe_evict[0] += 1
                st["msg"] = msg

            def emit_mm2(c):
                st = state.pop(c)
                pi, half = divmod(c, 2)
                pst = pair_state[pi]
                msg = pst["msg"]
                for t in range(TPC):
                    n_mm2[0] += 1
                    nc.tensor.matmul(
                        pooled[:],
                        st["mk"][:, t, :],
                        msg[:, half * TPC + t, :],
                        start=(n_mm2[0] == 1),
                        stop=(n_mm2[0] == N_CHUNKS * TPC),
                        skip_group_check=True,
                    )
                if half == 1:
                    del pair_state[pi]

            # stage order per index: oldest work first, so each engine's
            # in-order queue never has a young wait blocking old ready work.
            # The pair transpose runs TD chunks after the pair's second
            # chunk; each chunk's mm2 runs md (PE pairs) or md_dma (DMA
            # pairs, longer XBAR round trip) chunks after its front stage —
            # PSUM accumulation order is irrelevant, so start/stop follow
            # emission order via the n_mm2 counter.
            MDD = OPTS["md_dma"] if dtp else MD
            mm2_at = {}
            for c in range(N_CHUNKS):
                dl = c + (MDD if is_dma_pair(c // 2) else MD)
                # tail clamp: compress the post-loop mm2 backlog, but never
                # ahead of the pair's transpose emission (PE is in-order —
                # an mm2 enqueued before the transposes feeding it deadlocks)
                trans_idx = (c // 2) * 2 + 1 + TD
                dl = max(trans_idx + 1, min(dl, N_CHUNKS + 2))
                mm2_at.setdefault(dl, []).append(c)
            for cc in range(N_CHUNKS + MDD + 1):
                for c in mm2_at.get(cc, ()):
                    emit_mm2(c)
                tc_c = cc - TD          # second chunk of a pair at stage TD
                if TD <= cc and tc_c < N_CHUNKS and tc_c % 2 == 1:
                    emit_transpose(tc_c // 2)
                if cc < N_CHUNKS:
                    emit_front(cc)

            res = out_pool.tile([B, D], F32)
            nc.vector.tensor_copy(out=res[:], in_=pooled[:])
            nc.sync.dma_start(out_d, res[:])

    return nc


_CACHE: dict = {}


def _get_bass() -> bass.Bass:
    if "nc" not in _CACHE:
        _CACHE["nc"] = build_bass()
    return _CACHE["nc"]


def _prepare_in_maps(nodes, owner_masks, Wt, bt, Wg, bg):
    nodes_h = np.asarray(nodes, dtype=NP_F16)
    masks = np.asarray(owner_masks)
    np_mk = NP_F8 if OPTS["masks_fp8"] else NP_F16
    cst16 = np.concatenate(
        [
            np.asarray(Wg, dtype=NP_F16),
            np.asarray(Wt, dtype=NP_F16),
            np.eye(P, dtype=NP_F16),
        ],
        axis=1,
    )
    cst32 = np.stack(
        [np.asarray(bt, np.float32), np.asarray(bg, np.float32)], axis=1
    )
    cst16 = np.ascontiguousarray(cst16)
    cst32 = np.ascontiguousarray(cst32)

    in_maps = []
    for core in range(N_CORES):
        off = core * N_PER_CORE
        ncr = np.zeros((P, N_PAD), dtype=NP_F16)
        ncr[:, :N_PER_CORE] = nodes_h[off : off + N_PER_CORE].T
        mp = np.zeros((B, N_PAD), dtype=np.int8)
        mp[:, :N_PER_CORE] = masks[:, off : off + N_PER_CORE]
        mkt = np.ascontiguousarray(
            mp.reshape(B, N_TILES, P).transpose(2, 1, 0)
        ).astype(np_mk)
        in_maps.append(
            {
                "nodesT": ncr,
                "masksT": mkt,
                "cst16": cst16,
                "cst32": cst32,
            }
        )
    return in_maps


def run(inputs: dict, trace: bool = False):
    """Run the kernel. Returns (pooled [B, D] float32, BassKernelResults)."""
    nc = _get_bass()
    in_maps = _prepare_in_maps(**inputs)
    rb = run_bass_kernel_spmd(
        nc, in_maps, core_ids=list(range(N_CORES)), trace=trace
    )
    parts = np.stack([r["out"].astype(np.float64) for r in rb.results])
    pooled = parts.sum(axis=0)
    return pooled.astype(np.float32), rb


def kernel(**inputs) -> np.ndarray:
    try:
        out, _ = run(inputs, trace=False)
    except Exception:
        # transient device errors (e.g. residual bad state from a previous
        # crashed NEFF) have been observed once; one retry clears them
        out, _ = run(inputs, trace=False)
    return out


if __name__ == "__main__":
    rng = np.random.default_rng(0)
    demo = {
        "nodes": rng.standard_normal((N_TOTAL, S), dtype=np.float32),
        "owner_masks": rng.integers(0, 2, (B, N_TOTAL)).astype(np.int32),
        "Wt": rng.standard_normal((S, D), dtype=np.float32) * 0.09,
        "bt": rng.standard_normal(D).astype(np.float32) * 0.09,
        "Wg": rng.standard_normal((S, D), dtype=np.float32) * 0.09,
        "bg": rng.standard_normal(D).astype(np.float32) * 0.09,
    }
    out = kernel(**demo)
    print(out.shape, out.dtype, np.abs(out).mean())
